# revision 1
# baseline (speedup 1.0000x reference)
"""Trainium2 Bass kernel for DiffusionReturnPrediction.

Data-parallel over batch (B=1024 -> 128/core on 8 cores). Per core:
  phase 1: score-net GEMM1  h = silu(x_flat @ W1 + 0.1*ws + b1)   (bf16)
  phase 2: score-net GEMM2  scores = h @ W2 + b2, scattered (via PE
           transpose) into the LSTM's [feature, (n,b)] input layout
  phase 3: 180-step LSTM over B*N=1792 sequences, H=128
           z kept as [gate_unit(128) x bn] in PSUM, gate math on ACT/DVE
  phase 4: GCN (A baked as immediates) + MLP head + spatial pool

All matmuls bf16 (fp32 PSUM accumulation); final pool matmul fp32.
"""

import numpy as np
import ml_dtypes

import concourse.bacc as bacc
import concourse.bass as bass
import concourse.tile as tile
import concourse.mybir as mybir

BF16 = mybir.dt.bfloat16
F32 = mybir.dt.float32
AF = mybir.ActivationFunctionType

B, T, N, F = 1024, 180, 14, 4
D = N * T * F          # 10080
SNH = 1024
H = 128
G = 128
NOUT = 8
NCORES = 8
BS = B // NCORES       # 128 batch per core
BN = BS * N            # 1792
KT = (D + 1 + 127) // 128   # 79 k-tiles for GEMM1 (incl. ones row)
K_PAD = KT * 128
CH = 448               # LSTM bn-chunk width (4 chunks of 448)
NG2 = 6                # t-groups of 32 (180 -> 6 groups, last partial)


def _bf16(a):
    return np.ascontiguousarray(a, dtype=np.float32).astype(ml_dtypes.bfloat16)


def _f32(a):
    return np.ascontiguousarray(a, dtype=np.float32)


def build_nc(A_np, reps=1, t_steps=None, dumps=(), stop_after=None, zero_bias=False):
    """Build + compile the per-core Bass program. A_np: [14,14] f32 dense
    normalized adjacency (baked as immediates)."""
    if t_steps is None:
        t_steps = T
    nc = bacc.Bacc(None, target_bir_lowering=False)
    dump_es = {}
    for dn, dshape, ddt in (
        ("d_hsn", [128, SNH], BF16), ("d_hT", [128, SNH], BF16),
        ("d_xcomb", [128, 12 * BN], BF16),
        ("d_hbf", [H, BN], BF16), ("d_cbf", [H, BN], BF16),
        ("d_ubf", [G, BN], BF16), ("d_vm", [G // 2, BN], BF16),
        ("d_v1f", [1, BN], F32),
    ):
        if dn in dumps:
            dump_es[dn] = nc.declare_dram_parameter(dn, dshape, ddt, isOutput=True)

    xt_e = nc.declare_dram_parameter("xt", [128, KT * 128], BF16, isOutput=False)
    xstg_e = nc.declare_dram_parameter("xstage", [128, N * T * 8], BF16, isOutput=False)
    w1_e = nc.declare_dram_parameter("w1", [D + 1, SNH], BF16, isOutput=False)
    w2_e = nc.declare_dram_parameter("w2", [128, 42 * 8 * 256], BF16, isOutput=False)
    w2b_e = nc.declare_dram_parameter("w2bias", [1, D], BF16, isOutput=False)
    whh_e = nc.declare_dram_parameter("whh", [H, 4 * H], BF16, isOutput=False)
    wihc_e = nc.declare_dram_parameter("wihc", [128, 4 * 512], BF16, isOutput=False)
    bg_e = nc.declare_dram_parameter("bg", [H, 4], F32, isOutput=False)
    gcnw_e = nc.declare_dram_parameter("gcnw", [H, G], BF16, isOutput=False)
    gcnb_e = nc.declare_dram_parameter("gcnb", [G, 1], F32, isOutput=False)
    mlpw1_e = nc.declare_dram_parameter("mlpw1", [G, G // 2], BF16, isOutput=False)
    mlpb1_e = nc.declare_dram_parameter("mlpb1", [G // 2, 1], F32, isOutput=False)
    mlpw2_e = nc.declare_dram_parameter("mlpw2", [G // 2, 1], BF16, isOutput=False)
    mlpb2_e = nc.declare_dram_parameter("mlpb2", [1, 1], F32, isOutput=False)
    poolw_e = nc.declare_dram_parameter("poolw", [N + 1, NOUT], F32, isOutput=False)
    ident_e = nc.declare_dram_parameter("ident", [128, 128], BF16, isOutput=False)
    out_e = nc.declare_dram_parameter("out", [BS, NOUT], F32, isOutput=True)

    with tile.TileContext(nc) as tc:
        with tc.tile_pool(name="const", bufs=1) as cp:
            identt = cp.tile([128, 128], BF16)
            nc.sync.dma_start(identt[:], ident_e[:])
            bgt = cp.tile([H, 4], F32)
            nc.sync.dma_start(bgt[:], bg_e[:])
            gcnwt = cp.tile([H, G], BF16)
            nc.sync.dma_start(gcnwt[:], gcnw_e[:])
            gcnbt = cp.tile([G, 1], F32)
            nc.sync.dma_start(gcnbt[:], gcnb_e[:])
            mlpw1t = cp.tile([G, G // 2], BF16)
            nc.sync.dma_start(mlpw1t[:], mlpw1_e[:])
            mlpb1t = cp.tile([G // 2, 1], F32)
            nc.sync.dma_start(mlpb1t[:], mlpb1_e[:])
            mlpw2t = cp.tile([G // 2, 1], BF16)
            nc.sync.dma_start(mlpw2t[:], mlpw2_e[:])
            mlpb2t = cp.tile([1, 1], F32)
            nc.sync.dma_start(mlpb2t[:], mlpb2_e[:])
            poolwt = cp.tile([N + 1, NOUT], F32)
            nc.sync.dma_start(poolwt[:], poolw_e[:])
            whht = cp.tile([H, 4 * H], BF16)
            nc.sync.dma_start(whht[:], whh_e[:])
            wihct = cp.tile([128, 4 * 512], BF16)
            nc.sync.dma_start(wihct[:], wihc_e[:])
            ones1 = cp.tile([1, BS], BF16)
            nc.vector.memset(ones1[:], 1.0)

            # resident big tensors
            xstgt = cp.tile([128, N * T * 8], BF16)  # [b, (n,t,feat8)] staging
            nc.sync.dma_start(xstgt[:], xstg_e[:])
            NG16 = 12
            xcomb = cp.tile([128, NG16 * BN], BF16)  # [(t%16)*8+ff, (g16,n,b)]
            hT = cp.tile([128, SNH], BF16)          # transposed score-net hidden
            hbf = cp.tile([H, BN], BF16)            # LSTM h state
            cbf = cp.tile([H, BN], BF16)            # LSTM c state

            for _rep in range(reps):
                # ---------------- phase 1: GEMM1 ----------------
                with tc.tile_pool(name="p1", bufs=1) as p1, \
                     tc.tile_pool(name="w1p", bufs=8) as w1p, \
                     tc.tile_pool(name="ps1", bufs=1, space="PSUM") as ps1, \
                     tc.tile_pool(name="ps1t", bufs=2, space="PSUM") as ps1t:
                    xts = p1.tile([128, KT * 128], BF16, tag="xts")
                    nc.sync.dma_start(xts[:], xt_e[:])
                    hps = ps1.tile([128, SNH], F32)
                    for k in range(KT):
                        rows = min(128, D + 1 - k * 128)
                        w1t = w1p.tile([128, SNH], BF16, tag="w1t")
                        nc.sync.dma_start(w1t[0:rows, :],
                                          w1_e[k * 128:k * 128 + rows, :])
                        for jg in range(2):
                            nc.tensor.matmul(
                                hps[:, jg * 512:(jg + 1) * 512],
                                xts[0:rows, k * 128:k * 128 + 128],
                                w1t[0:rows, jg * 512:(jg + 1) * 512],
                                start=(k == 0), stop=(k == KT - 1))
                    hsn = p1.tile([128, SNH], BF16, tag="hsn")
                    nc.scalar.activation(hsn[:], hps[:], AF.Silu)
                    for j in range(8):
                        tp = ps1t.tile([128, 128], BF16, tag="tp1")
                        nc.tensor.transpose(tp[:], hsn[:, j * 128:(j + 1) * 128],
                                            identt[:])
                        nc.vector.tensor_copy(hT[:, j * 128:(j + 1) * 128], tp[:])
                    if "d_hsn" in dump_es:
                        nc.sync.dma_start(dump_es["d_hsn"][:, :], hsn[:])
                    if "d_hT" in dump_es:
                        nc.sync.dma_start(dump_es["d_hT"][:, :], hT[:])

                if stop_after == "p1":
                    continue
                # ---------------- phase 2: GEMM2 + scatter ----------------
                with tc.tile_pool(name="p2", bufs=3) as p2, \
                     tc.tile_pool(name="ps2", bufs=2, space="PSUM") as ps2, \
                     tc.tile_pool(name="ps2t", bufs=2, space="PSUM") as ps2t:
                    for n in range(N):
                        for ci, (c0, W) in enumerate(((0, 256), (256, 256),
                                                      (512, 208))):
                            blk = (n * 3 + ci) * 8 * 256
                            w2t = p2.tile([128, 8 * 256], BF16, tag="w2t")
                            nc.sync.dma_start(w2t[:],
                                              w2_e[:, blk:blk + 8 * 256])
                            w2b = p2.tile([1, 256], BF16, tag="w2b")
                            nc.sync.dma_start(
                                w2b[:, 0:W],
                                w2b_e[0:1, n * 720 + c0:n * 720 + c0 + W])
                            sc = ps2.tile([128, 256], F32, tag="sc")
                            for k in range(8):
                                nc.tensor.matmul(
                                    sc[:, 0:W], hT[:, k * 128:(k + 1) * 128],
                                    w2t[:, k * 256:k * 256 + W],
                                    start=(k == 0), stop=False)
                            nc.tensor.matmul(sc[:, 0:W], ones1[0:1, :],
                                             w2b[0:1, 0:W], start=False, stop=True)
                            Wt = W // 4
                            t0 = c0 // 4
                            scv = sc.rearrange("p (t f) -> p t f", f=4)
                            xsv = xstgt.rearrange("p (n t e) -> p n t e",
                                                  t=T, e=8)
                            nc.scalar.copy(xsv[:, n, t0:t0 + Wt, 4:8],
                                           scv[:, 0:Wt, :])
                        # transposes: 16-t blocks -> xcomb
                        for tb in range(12):
                            cols = 128 if tb < 11 else 32
                            tp2 = ps2t.tile([128, 128], BF16, tag="tp2")
                            nc.tensor.transpose(
                                tp2[0:cols, :],
                                xstgt[:, n * 1440 + tb * 128:
                                      n * 1440 + tb * 128 + cols],
                                identt[:])
                            nc.vector.tensor_copy(
                                xcomb[0:cols, (tb * N + n) * 128:
                                      (tb * N + n) * 128 + 128],
                                tp2[0:cols, :])

                if stop_after == "p2":
                    continue
                # ---------------- phase 3: LSTM ----------------
                nc.gpsimd.memset(hbf[:], 0.0)
                nc.gpsimd.memset(cbf[:], 0.0)
                with tc.tile_pool(name="p3", bufs=3) as p3, \
                     tc.tile_pool(name="zp", bufs=1, space="PSUM") as zp:
                    zps = zp.tile([128, 4096], F32)
                    for t in range(t_steps):
                        g16 = t // 16
                        q0 = 32 * ((t % 16) // 4)
                        v = t % 4
                        for half in range(2):
                            cs = (2 * half, 2 * half + 1)
                            # combined input part (x + scores)
                            for g in range(4):
                                for c in cs:
                                    st = c % 2
                                    bk = st * 4 + (0, 1, 3, 2)[g]
                                    ps = zps[:, bk * 512:bk * 512 + CH]
                                    nc.tensor.matmul(
                                        ps,
                                        wihct[q0:q0 + 32,
                                              v * 512 + g * 128:
                                              v * 512 + (g + 1) * 128],
                                        xcomb[q0:q0 + 32,
                                              g16 * BN + c * CH:
                                              g16 * BN + c * CH + CH],
                                        start=True, stop=False,
                                        tile_position=(q0, 0))
                            # recurrent part
                            for g in range(4):
                                for c in cs:
                                    st = c % 2
                                    bk = st * 4 + (0, 1, 3, 2)[g]
                                    ps = zps[:, bk * 512:bk * 512 + CH]
                                    nc.tensor.matmul(
                                        ps, whht[:, g * 128:(g + 1) * 128],
                                        hbf[:, c * CH:c * CH + CH],
                                        start=False, stop=True)
                            # gate math per chunk
                            zv = zps.rearrange("p (b e) -> p b e", e=512)
                            ots = []
                            for c in cs:
                                st = c % 2
                                def zsl(g):
                                    return zps[:, (st * 4 + g) * 512:
                                               (st * 4 + g) * 512 + CH]
                                gt = p3.tile([128, CH], BF16, tag="gt")
                                if zero_bias:
                                    ifo = p3.tile([128, 3 * CH], BF16, tag="ifo")
                                    ifov = ifo.rearrange("p (b e) -> p b e", e=CH)
                                    nc.scalar.activation(
                                        ifov[:, 0:3, :],
                                        zv[:, st * 4:st * 4 + 3, 0:CH],
                                        AF.Sigmoid)
                                    nc.scalar.activation(gt[:], zsl(3), AF.Tanh)
                                    it = ifo[:, 0:CH]
                                    ft = ifo[:, CH:2 * CH]
                                    ot = ifo[:, 2 * CH:3 * CH]
                                else:
                                    itt = p3.tile([128, CH], BF16, tag="it")
                                    ftt = p3.tile([128, CH], BF16, tag="ft")
                                    ott = p3.tile([128, CH], BF16, tag="ot")
                                    nc.scalar.activation(itt[:], zsl(0), AF.Sigmoid,
                                                         bias=bgt[:, 0:1])
                                    nc.scalar.activation(gt[:], zsl(3), AF.Tanh,
                                                         bias=bgt[:, 2:3])
                                    nc.scalar.activation(ftt[:], zsl(1), AF.Sigmoid,
                                                         bias=bgt[:, 1:2])
                                    nc.scalar.activation(ott[:], zsl(2), AF.Sigmoid,
                                                         bias=bgt[:, 3:4])
                                    it, ft, ot = itt[:], ftt[:], ott[:]
                                ig = p3.tile([128, CH], BF16, tag="ig")
                                fc = p3.tile([128, CH], BF16, tag="fc")
                                nc.vector.tensor_mul(ig[:], it, gt[:])
                                nc.vector.tensor_mul(fc[:], ft,
                                                     cbf[:, c * CH:c * CH + CH])
                                nc.vector.tensor_add(cbf[:, c * CH:c * CH + CH],
                                                     ig[:], fc[:])
                                ots.append(ot)
                            # tanh(c) + h for the whole half (896 wide)
                            h0 = cs[0] * CH
                            tct = p3.tile([128, 2 * CH], BF16, tag="tct")
                            nc.scalar.activation(tct[:],
                                                 cbf[:, h0:h0 + 2 * CH],
                                                 AF.Tanh)
                            for ci, c in enumerate(cs):
                                nc.vector.tensor_mul(
                                    hbf[:, c * CH:c * CH + CH],
                                    ots[ci], tct[:, ci * CH:(ci + 1) * CH])

                if stop_after == "lstm":
                    continue
                for dn, src in (("d_xcomb", xcomb),
                                ("d_hbf", hbf), ("d_cbf", cbf)):
                    if dn in dump_es:
                        nc.sync.dma_start(dump_es[dn][:, :], src[:])
                # ---------------- phase 4: GCN + MLP + pool ----------------
                with tc.tile_pool(name="p4", bufs=2) as p4, \
                     tc.tile_pool(name="ps4", bufs=2, space="PSUM") as ps4:
                    ubf = p4.tile([G, BN], BF16, tag="ubf")
                    for c in range(4):
                        ups = ps4.tile([G, CH], F32, tag="ups")
                        nc.tensor.matmul(ups[:], gcnwt[:],
                                         hbf[:, c * CH:c * CH + CH],
                                         start=True, stop=True)
                        nc.vector.tensor_scalar(
                            out=ubf[:, c * CH:c * CH + CH], in0=ups[:],
                            scalar1=gcnbt[:, 0:1], scalar2=None,
                            op0=mybir.AluOpType.add)
                    vbf = p4.tile([G // 2, BN], BF16, tag="vbf")
                    for c in range(4):
                        vps = ps4.tile([G // 2, CH], F32, tag="vps")
                        nc.tensor.matmul(vps[:], mlpw1t[:],
                                         ubf[:, c * CH:c * CH + CH],
                                         start=True, stop=True)
                        nc.scalar.copy(vbf[:, c * CH:c * CH + CH], vps[:])
                    # A-mix over nodes (A baked as immediates, sparse)
                    vm = p4.tile([G // 2, BN], BF16, tag="vm")
                    tmpm = p4.tile([G // 2, 128], BF16, tag="tmpm")
                    for n in range(N):
                        js = [j for j in range(N) if A_np[n, j] != 0.0]
                        j0 = js[0]
                        nc.vector.tensor_scalar(
                            out=vm[:, n * 128:(n + 1) * 128],
                            in0=vbf[:, j0 * 128:(j0 + 1) * 128],
                            scalar1=float(A_np[n, j0]), scalar2=None,
                            op0=mybir.AluOpType.mult)
                        for j in js[1:]:
                            nc.vector.tensor_scalar(
                                out=tmpm[:],
                                in0=vbf[:, j * 128:(j + 1) * 128],
                                scalar1=float(A_np[n, j]), scalar2=None,
                                op0=mybir.AluOpType.mult)
                            nc.vector.tensor_add(
                                vm[:, n * 128:(n + 1) * 128],
                                vm[:, n * 128:(n + 1) * 128], tmpm[:])
                    hid = p4.tile([G // 2, BN], BF16, tag="hid")
                    nc.scalar.activation(hid[:], vm[:], AF.Silu,
                                         bias=mlpb1t[:, 0:1])
                    v1f = p4.tile([1, BN], F32, tag="v1f")
                    for c in range(4):
                        ohps = ps4.tile([1, CH], F32, tag="ohps")
                        nc.tensor.matmul(ohps[:], mlpw2t[:],
                                         hid[:, c * CH:c * CH + CH],
                                         start=True, stop=True)
                        nc.vector.tensor_scalar(
                            out=v1f[:, c * CH:c * CH + CH], in0=ohps[:],
                            scalar1=mlpb2t[0:1, 0:1], scalar2=None,
                            op0=mybir.AluOpType.add)
                    v15 = p4.tile([N + 1, BS], F32, tag="v15")
                    nc.vector.memset(v15[:], 1.0)
                    for n in range(N):
                        nc.sync.dma_start(v15[n:n + 1, :],
                                          v1f[0:1, n * BS:(n + 1) * BS])
                    fps = ps4.tile([NOUT, BS], F32, tag="fps")
                    nc.tensor.matmul(fps[:], poolwt[:], v15[:],
                                     start=True, stop=True)
                    outsb = p4.tile([NOUT, BS], F32, tag="outsb")
                    nc.vector.tensor_copy(outsb[:], fps[:])
                    for o in range(NOUT):
                        nc.sync.dma_start(out_e[:, o:o + 1],
                                          outsb[o:o + 1, :])
                    for dn, src in (("d_ubf", ubf), ("d_vm", vm),
                                    ("d_v1f", v1f)):
                        if dn in dump_es:
                            nc.sync.dma_start(dump_es[dn][:, :], src[:])

    nc.compile()
    return nc


def make_adjacency(edge_index):
    ei = np.asarray(edge_index)
    loops = np.arange(N, dtype=ei.dtype)
    row = np.concatenate([ei[0], loops])
    col = np.concatenate([ei[1], loops])
    deg = np.zeros(N, np.float32)
    np.add.at(deg, col, 1.0)
    dinv = np.where(deg > 0, deg ** -0.5, 0.0).astype(np.float32)
    norm = dinv[row] * dinv[col]
    A = np.zeros((N, N), np.float32)
    np.add.at(A, (col, row), norm)
    return A


def prep_inputs(inputs):
    """Host-side prep: per-core shards + weight layouts. Returns in_maps."""
    x = np.asarray(inputs["x"], np.float32)
    A = make_adjacency(inputs["edge_index"])
    c1 = 0.1 * np.asarray(inputs["sn_ws"], np.float32) + \
        np.asarray(inputs["sn_b1"], np.float32)
    W1p = np.asarray(inputs["sn_W1"], np.float32).reshape(N, T, F, SNH) \
        .transpose(1, 0, 2, 3).reshape(D, SNH)
    w1 = _bf16(np.vstack([W1p, c1[None, :]]))
    W2f = np.asarray(inputs["sn_W2"], np.float32)          # [1024, 10080]
    w2img = np.zeros((128, 42 * 8 * 256), np.float32)
    for n in range(N):
        for ci, (c0, W) in enumerate(((0, 256), (256, 256), (512, 208))):
            blk = (n * 3 + ci) * 8 * 256
            for k in range(8):
                w2img[:, blk + k * 256:blk + k * 256 + W] = \
                    W2f[k * 128:(k + 1) * 128, n * 720 + c0:n * 720 + c0 + W]
    w2 = _bf16(w2img)
    w2bias = _bf16(np.asarray(inputs["sn_b2"], np.float32).reshape(1, D))
    wih = np.asarray(inputs["lstm_Wih"], np.float32).T      # [8, 512]
    whh = _bf16(np.asarray(inputs["lstm_Whh"], np.float32).T)  # [128, 512]
    wihc32 = np.zeros((32, 4, 512), np.float32)
    for v in range(4):
        wihc32[v * 8:v * 8 + 8, v, :] = wih          # [8, 512] x+s stacked
    wihc = _bf16(np.tile(wihc32.reshape(32, 4 * 512), (4, 1)))
    bg = _f32((np.asarray(inputs["lstm_bih"], np.float32) +
               np.asarray(inputs["lstm_bhh"], np.float32))
              .reshape(4, H).T)                              # [128, 4]
    gcnw = _bf16(inputs["gcn_W"])
    gcnb = _f32(np.asarray(inputs["gcn_b"]).reshape(G, 1))
    mlpw1 = _bf16(inputs["mlp_W1"])
    mlpb1 = _f32(np.asarray(inputs["mlp_b1"]).reshape(G // 2, 1))
    mlpw2 = _bf16(inputs["mlp_W2"])
    mlpb2 = _f32(np.asarray(inputs["mlp_b2"]).reshape(1, 1))
    poolw = _f32(np.vstack([np.asarray(inputs["pool_W"], np.float32),
                            np.asarray(inputs["pool_b"], np.float32)[None, :]]))
    ident = _bf16(np.eye(128, dtype=np.float32))

    shared = dict(w1=w1, w2=w2, w2bias=w2bias, whh=whh, wihc=wihc, bg=bg,
                  gcnw=gcnw, gcnb=gcnb, mlpw1=mlpw1, mlpb1=mlpb1,
                  mlpw2=mlpw2, mlpb2=mlpb2, poolw=poolw, ident=ident)
    in_maps = []
    for cidx in range(NCORES):
        xc = x[cidx * BS:(cidx + 1) * BS]            # [128, T, N, F]
        xflat = xc.reshape(BS, D)                    # (t,n,f) order
        xT = np.vstack([xflat.T, np.ones((1, BS), np.float32)])
        xTpad = np.zeros((K_PAD, BS), np.float32)
        xTpad[:D + 1] = xT
        xT = xTpad.reshape(KT, 128, BS).transpose(1, 0, 2).reshape(128, KT * BS)
        xstage = np.zeros((BS, N, T, 8), np.float32)
        xstage[:, :, :, 0:4] = xc.transpose(0, 2, 1, 3)
        xstage = xstage.reshape(BS, N * T * 8)
        in_maps.append(dict(xt=_bf16(xT), xstage=_bf16(xstage), **shared))
    return in_maps, A


def kernel(**inputs):
    from concourse.bass_utils import run_bass_kernel_spmd
    in_maps, A = prep_inputs(inputs)
    zb = not (np.any(np.asarray(inputs["lstm_bih"])) or
              np.any(np.asarray(inputs["lstm_bhh"])))
    nc = build_nc(A, reps=1, zero_bias=zb)
    res = run_bass_kernel_spmd(nc, in_maps, core_ids=list(range(NCORES)))
    out = np.concatenate([res.results[c]["out"] for c in range(NCORES)], axis=0)
    return out.astype(np.float32)

